# revision 23
# baseline (speedup 1.0000x reference)
"""Distributed attention kernel for Trainium2 (8 NeuronCores).

Problem: softmax(Q @ K.T / sqrt(S)) @ V with S=8192, D=256, fp32 I/O.
Note the reference scales by sqrt(K.shape[-2]) = sqrt(S), NOT sqrt(D).

Sharding: Q rows split across 8 cores (1024 rows each); K, V replicated.
No collectives needed - each core computes its output rows independently.

Per-core algorithm (BF16 matmuls, FP32 accumulation):
  - DMA-load Q, K, V with inline f32->bf16 cast (SWDGE).
  - PE-transpose Q and K into Q^T [D, 1024] and K^T [D, S] (the score
    matmul contracts over D, so both operands need D on partitions).
  - Scores are computed TRANSPOSED: S^T[keys, q] = K @ Q^T. The exp output
    P^T = exp(S^T * scale) (bf16, in SBUF) is then directly usable as the
    stationary operand (lhsT) of the P @ V matmul - no P transpose needed.
  - Scores are ~N(0, 1/32) so softmax needs no max subtraction.
  - V gets a ones-column appended: P @ [V | 1] accumulates both the
    unnormalized output and the softmax denominator in one PSUM chain.
  - Normalize with a per-partition reciprocal multiply, DMA out fp32.
"""

import numpy as np

S = 8192
D = 256
N_CORES = 8
SHARD = S // N_CORES  # 1024 query rows per core

_CACHE = {}


def _build(repeat=1, variant="stag"):
    import concourse.mybir as mybir
    import concourse.tile as tile
    from concourse import bacc
    from concourse.masks import make_identity

    f32 = mybir.dt.float32
    bf16 = mybir.dt.bfloat16
    f8 = mybir.dt.float8e4
    AF = mybir.ActivationFunctionType

    SCALE = 1.0 / float(np.sqrt(np.float32(S)))

    NKB = S // 128      # 64 key blocks
    NQC = SHARD // 512  # 2 query chunks per core
    NQT = 4             # 128-row query tiles per chunk

    nc = bacc.Bacc()
    q_ext = nc.dram_tensor("Q", [SHARD, D], f32, kind="ExternalInput")
    k_ext = nc.dram_tensor("K", [S, D], f32, kind="ExternalInput")
    v_ext = nc.dram_tensor("V", [S, D], f32, kind="ExternalInput")
    out_ext = nc.dram_tensor("out", [SHARD, D], f32, kind="ExternalOutput")

    with tile.TileContext(nc) as tc:
        with (
            tc.tile_pool(name="singles", bufs=1) as singles,
            tc.tile_pool(name="kstage", bufs=3) as kstage,
            tc.tile_pool(name="ptp", bufs=1) as ptp,
            tc.tile_pool(name="outp", bufs=4) as outp,
            tc.tile_pool(name="stp", bufs=2, space="PSUM") as stp,
            tc.tile_pool(
                name="pvp",
                bufs=2 if variant in ("seq", "seqp", "stag") else 4,
                space="PSUM",
            ) as pvp,
            tc.tile_pool(name="trp", bufs=2, space="PSUM") as trp,
        ):
            ident = singles.tile([128, 128], bf16, tag="ident", name="ident")
            make_identity(nc, ident)
            for _rep in range(repeat):
                _emit_body(nc, tc, singles, kstage, ptp, outp, stp, pvp,
                           trp, ident, q_ext, k_ext, v_ext, out_ext,
                           mybir, SCALE, NKB, NQC, NQT, variant)

    nc.finalize()
    return nc


def _emit_body(nc, tc, singles, kstage, ptp, outp, stp, pvp,
               trp, ident, q_ext, k_ext, v_ext, out_ext,
               mybir, SCALE, NKB, NQC, NQT, variant="stag"):
    f32 = mybir.dt.float32
    bf16 = mybir.dt.bfloat16
    f8 = mybir.dt.float8e4
    AF = mybir.ActivationFunctionType

    if True:
        if True:
            # ---- Q: load (cast to fp8) + PE-transpose into QT [128,2,SHARD]
            # QT[p, c, q] = Q[q, c*128+p]; dim1 = d-chunk for DoubleRow's
            # (p, t) contraction layout.
            qs = singles.tile([128, SHARD // 128, D], bf16, tag="qs", name="qs")
            q_re = q_ext.rearrange("(n p) d -> p n d", p=128)
            nc.gpsimd.dma_start(out=qs[:, 0:4, :], in_=q_re[:, 0:4, :])
            nc.gpsimd.dma_start(out=qs[:, 4:8, :], in_=q_re[:, 4:8, :])
            st_dt = bf16 if variant == "seqbf" else f8
            qt8 = singles.tile([128, 2, SHARD], st_dt, tag="qt8", name="qt8")
            for c in range(2):
                for g in range(SHARD // 512):
                    tr = (trp.tile([128, 512], bf16, tag="tr", name="tr")
                          if variant in ("seq", "seqp", "stag") else
                          stp.tile([128, 512], bf16, tag="st", name="tr"))
                    for j in range(4):
                        qi = g * 4 + j
                        nc.tensor.transpose(
                            tr[:, j * 128:(j + 1) * 128],
                            qs[:, qi, c * 128:(c + 1) * 128],
                            ident,
                        )
                    nc.vector.tensor_copy(
                        qt8[:, c, g * 512:(g + 1) * 512], tr[:]
                    )

            # ---- K: load (cast to fp8) + PE-transpose into KT [128,2,S] ----
            kt8 = singles.tile([128, 2, S], st_dt, tag="kt8", name="kt8")
            k_all = k_ext.rearrange("(b p) d -> p b d", p=128)
            groups = [(0, 4), (4, 4)] + [(8 * t, 8) for t in range(1, 8)]
            for b_start, nb in groups:
                ks = kstage.tile([128, 8, D], bf16, tag="ks", name="ks")
                nc.gpsimd.dma_start(
                    out=ks[:, 0:nb, :],
                    in_=k_all[:, b_start:b_start + nb, :],
                )
                for c in range(2):
                    for g in range(nb // 4):
                        tr = (trp.tile([128, 512], bf16, tag="tr", name="tr")
                          if variant in ("seq", "seqp", "stag") else
                          stp.tile([128, 512], bf16, tag="st", name="tr"))
                        for j in range(4):
                            n = g * 4 + j
                            nc.tensor.transpose(
                                tr[:, j * 128:(j + 1) * 128],
                                ks[:, n, c * 128:(c + 1) * 128],
                                ident,
                            )
                        b0 = b_start + g * 4
                        nc.vector.tensor_copy(
                            kt8[:, c, b0 * 128:(b0 + 4) * 128], tr[:]
                        )

            # ---- V: load (cast) with a ones-column appended ----
            vo = []
            v_re = v_ext.rearrange("(t n p) d -> t p n d", p=128, n=4)
            for t in range(S // 512):
                vt = singles.tile([128, 4, D + 1], bf16, tag=f"vo{t}", name=f"vo{t}")
                nc.vector.memset(vt[:, :, D:D + 1], 1.0)
                nc.gpsimd.dma_start(out=vt[:, :, 0:D], in_=v_re[t])
                vo.append(vt)

            # ---- main: ST stream interleaved with kb-major PV chains ----
            # PV for key block kb is emitted LAG key-block-pairs after its
            # exp, so the PE always has PV matmuls to run between ST
            # matmuls instead of stalling at the exp production rate.
            pts = {0: [], 1: []}
            chains = {}

            def st_step(qc, kbp):
                st = stp.tile([128, 1024], f32, tag="st", name="st")
                ptag = (f"pt{qc}_{kbp}" if variant in ("seqp", "stag")
                        else f"pt{kbp}")
                pt = ptp.tile([128, 1024], bf16, tag=ptag, name=ptag)
                for half in range(2):
                    kb = kbp * 2 + half
                    if variant == "seqbf":
                        for c in range(2):
                            nc.tensor.matmul(
                                st[:, half * 512:(half + 1) * 512],
                                kt8[:, c, kb * 128:(kb + 1) * 128],
                                qt8[:, c, qc * 512:(qc + 1) * 512],
                                start=(c == 0),
                                stop=(c == 1),
                            )
                    else:
                        nc.tensor.matmul(
                            st[:, half * 512:(half + 1) * 512],
                            kt8[:, :, kb * 128:(kb + 1) * 128],
                            qt8[:, :, qc * 512:(qc + 1) * 512],
                            start=True,
                            stop=True,
                            perf_mode=mybir.MatmulPerfMode.DoubleRow,
                        )
                nc.scalar.activation(pt[:], st[:], AF.Exp, scale=SCALE)
                pts[qc].append(pt)

            def pv_kb(qc, kb):
                for qt_i in range(NQT):
                    if kb == 0:
                        chains[(qc, qt_i)] = pvp.tile(
                            [128, D + 1], f32, tag="pv", name="pv"
                        )
                    pv = chains[(qc, qt_i)]
                    col0 = (kb % 2) * 512 + qt_i * 128
                    nc.tensor.matmul(
                        pv[:],
                        pts[qc][kb // 2][:, col0:col0 + 128],
                        vo[kb // 4][:, kb % 4, :],
                        start=(kb == 0),
                        stop=(kb == NKB - 1),
                    )
                if kb == NKB - 1:
                    for qt_i in range(NQT):
                        pv = chains[(qc, qt_i)]
                        rcp = outp.tile([128, 1], f32, tag="rcp", name="rcp")
                        nc.vector.reciprocal(rcp[:], pv[:, D:D + 1])
                        ot = outp.tile([128, D], f32, tag="ot", name="ot")
                        nc.vector.tensor_scalar_mul(ot[:], pv[:, 0:D], rcp[:])
                        row0 = qc * 512 + qt_i * 128
                        nc.sync.dma_start(
                            out=out_ext[row0:row0 + 128, :], in_=ot[:]
                        )

            if variant == "ilv":
                LAG = 2
                BURST = 8  # drain PV in bursts, limits LDW mode flips
                queue = []
                started = 0
                for qc in range(NQC):
                    for kbp in range(NKB // 2):
                        st_step(qc, kbp)
                        started += 1
                        queue.append((qc, 2 * kbp))
                        queue.append((qc, 2 * kbp + 1))
                        if started > LAG and len(queue) >= 2 * LAG + BURST:
                            for _ in range(BURST):
                                pv_kb(*queue.pop(0))
                while queue:
                    pv_kb(*queue.pop(0))
            elif variant == "stag":
                # qc0: plain ST phase. Then steps interleaving one ST(qc1)
                # pair with 8 contiguous PV(qc0) chain matmuls (chains stay
                # qt-major / single-bank). Finally PV(qc1) qt-major.
                def pv_chain_part(qc, qt_i, kb0, kb1):
                    if kb0 == 0:
                        chains[(qc, qt_i)] = pvp.tile(
                            [128, D + 1], f32, tag="pv", name="pv"
                        )
                    pv = chains[(qc, qt_i)]
                    for kb in range(kb0, kb1):
                        col0 = (kb % 2) * 512 + qt_i * 128
                        nc.tensor.matmul(
                            pv[:],
                            pts[qc][kb // 2][:, col0:col0 + 128],
                            vo[kb // 4][:, kb % 4, :],
                            start=(kb == 0),
                            stop=(kb == NKB - 1),
                        )
                    if kb1 == NKB:
                        rcp = outp.tile([128, 1], f32, tag="rcp", name="rcp")
                        nc.vector.reciprocal(rcp[:], pv[:, D:D + 1])
                        ot = outp.tile([128, D], f32, tag="ot", name="ot")
                        nc.vector.tensor_scalar_mul(ot[:], pv[:, 0:D], rcp[:])
                        row0 = qc * 512 + qt_i * 128
                        nc.sync.dma_start(
                            out=out_ext[row0:row0 + 128, :], in_=ot[:]
                        )

                for kbp in range(NKB // 2):
                    st_step(0, kbp)
                for step in range(32):
                    st_step(1, step)
                    qt_i, seg = divmod(step, 8)
                    pv_chain_part(0, qt_i, seg * 8, seg * 8 + 8)
                for qt_i in range(NQT):
                    pv_chain_part(1, qt_i, 0, NKB)
            elif variant == "seq4":
                # per-chunk: all ST, then kb-major PV (all 4 chains advance
                # together so pt slots release early for the next chunk)
                for qc in range(NQC):
                    for kbp in range(NKB // 2):
                        st_step(qc, kbp)
                    for kb in range(NKB):
                        pv_kb(qc, kb)
            else:
                # sequential: all ST of a chunk, then qt-major PV chains
                for qc in range(NQC):
                    for kbp in range(NKB // 2):
                        st_step(qc, kbp)
                    for qt_i in range(NQT):
                        pv = pvp.tile([128, D + 1], f32, tag="pv", name="pv")
                        for kb in range(NKB):
                            col0 = (kb % 2) * 512 + qt_i * 128
                            nc.tensor.matmul(
                                pv[:],
                                pts[qc][kb // 2][:, col0:col0 + 128],
                                vo[kb // 4][:, kb % 4, :],
                                start=(kb == 0),
                                stop=(kb == NKB - 1),
                            )
                        rcp = outp.tile([128, 1], f32, tag="rcp", name="rcp")
                        nc.vector.reciprocal(rcp[:], pv[:, D:D + 1])
                        ot = outp.tile([128, D], f32, tag="ot", name="ot")
                        nc.vector.tensor_scalar_mul(ot[:], pv[:, 0:D], rcp[:])
                        row0 = qc * 512 + qt_i * 128
                        nc.sync.dma_start(
                            out=out_ext[row0:row0 + 128, :], in_=ot[:]
                        )


def _get_nc(repeat=1, variant="stag"):
    key = f"nc{repeat}-{variant}"
    if key not in _CACHE:
        _CACHE[key] = _build(repeat, variant)
    return _CACHE[key]


def run(inputs, trace=False):
    """Run on 8 cores; returns (full_output, BassKernelResults)."""
    from concourse.bass_utils import run_bass_kernel_spmd

    Q = np.ascontiguousarray(np.asarray(inputs["Q"], dtype=np.float32))
    K = np.ascontiguousarray(np.asarray(inputs["K"], dtype=np.float32))
    V = np.ascontiguousarray(np.asarray(inputs["V"], dtype=np.float32))

    nc = _get_nc()
    in_maps = [
        {"Q": Q[i * SHARD:(i + 1) * SHARD], "K": K, "V": V}
        for i in range(N_CORES)
    ]
    res = run_bass_kernel_spmd(
        nc, in_maps, core_ids=list(range(N_CORES)), trace=trace
    )
    out = np.concatenate([res.results[i]["out"] for i in range(N_CORES)], axis=0)
    return out, res


def kernel(**inputs) -> np.ndarray:
    out, _ = run(inputs, trace=False)
    return out


# revision 24
# speedup vs baseline: 6.6892x; 6.6892x over previous
"""Distributed attention kernel for Trainium2 (8 NeuronCores).

Problem: softmax(Q @ K.T / sqrt(S)) @ V with S=8192, D=256, fp32 I/O.
Note the reference scales by sqrt(K.shape[-2]) = sqrt(S), NOT sqrt(D).

Sharding: Q rows split across 8 cores (1024 rows each); K, V replicated.
No collectives needed - each core computes its output rows independently.

Per-core algorithm:
  - DMA-load Q, K, V with inline f32->bf16 cast (SWDGE).
  - PE-transpose Q and K into Q^T [D, 1024] and K^T [D, S] (the score
    matmul contracts over D, so both operands need D on partitions); the
    PSUM->SBUF copies cast them to fp8e4m3, laid out [128, 2, n] for
    DoubleRow's (partition, k-tile) contraction.
  - Scores are computed TRANSPOSED in one fp8 DoubleRow matmul per key
    block (contraction 256 at 2 MACs/cell/cycle): S^T[keys, q] = K @ Q^T.
    The exp output P^T = exp(S^T * scale) (bf16, in SBUF) is directly the
    stationary operand (lhsT) of the P @ V matmul - no P transpose needed.
  - Scores are ~N(0, 1/32) so softmax needs no max subtraction; fp8
    score error (~0.7% on softmax weights) dominates the output rel_err
    of ~7e-3, well under the 2e-2 gate.
  - V (bf16) gets a ones-column appended: P @ [V | 1] accumulates the
    unnormalized output and the softmax denominator in one PSUM chain.
  - Normalize with a per-partition reciprocal multiply, DMA out fp32.
  - Schedule ("stag"): chunk-0 scores, then steps interleaving one
    chunk-1 score matmul with 8 contiguous chunk-0 PV chain matmuls
    (PV chains stay single-PSUM-bank - HW-measured faster than
    multi-chain interleaves), then chunk-1 PV.
"""

import numpy as np

S = 8192
D = 256
N_CORES = 8
SHARD = S // N_CORES  # 1024 query rows per core

_CACHE = {}


def _build(repeat=1, variant="stag"):
    import concourse.mybir as mybir
    import concourse.tile as tile
    from concourse import bacc
    from concourse.masks import make_identity

    f32 = mybir.dt.float32
    bf16 = mybir.dt.bfloat16
    f8 = mybir.dt.float8e4
    AF = mybir.ActivationFunctionType

    SCALE = 1.0 / float(np.sqrt(np.float32(S)))

    NKB = S // 128      # 64 key blocks
    NQC = SHARD // 512  # 2 query chunks per core
    NQT = 4             # 128-row query tiles per chunk

    nc = bacc.Bacc()
    q_ext = nc.dram_tensor("Q", [SHARD, D], f32, kind="ExternalInput")
    k_ext = nc.dram_tensor("K", [S, D], f32, kind="ExternalInput")
    v_ext = nc.dram_tensor("V", [S, D], f32, kind="ExternalInput")
    out_ext = nc.dram_tensor("out", [SHARD, D], f32, kind="ExternalOutput")

    with tile.TileContext(nc) as tc:
        with (
            tc.tile_pool(name="singles", bufs=1) as singles,
            tc.tile_pool(name="kstage", bufs=3) as kstage,
            tc.tile_pool(name="ptp", bufs=1) as ptp,
            tc.tile_pool(name="outp", bufs=4) as outp,
            tc.tile_pool(name="stp", bufs=2, space="PSUM") as stp,
            tc.tile_pool(
                name="pvp",
                bufs=2 if variant in ("seq", "seqp", "stag") else 4,
                space="PSUM",
            ) as pvp,
            tc.tile_pool(name="trp", bufs=2, space="PSUM") as trp,
        ):
            ident = singles.tile([128, 128], bf16, tag="ident", name="ident")
            make_identity(nc, ident)
            for _rep in range(repeat):
                _emit_body(nc, tc, singles, kstage, ptp, outp, stp, pvp,
                           trp, ident, q_ext, k_ext, v_ext, out_ext,
                           mybir, SCALE, NKB, NQC, NQT, variant)

    nc.finalize()
    return nc


def _emit_body(nc, tc, singles, kstage, ptp, outp, stp, pvp,
               trp, ident, q_ext, k_ext, v_ext, out_ext,
               mybir, SCALE, NKB, NQC, NQT, variant="stag"):
    f32 = mybir.dt.float32
    bf16 = mybir.dt.bfloat16
    f8 = mybir.dt.float8e4
    AF = mybir.ActivationFunctionType

    if True:
        if True:
            # ---- Q: load (cast to fp8) + PE-transpose into QT [128,2,SHARD]
            # QT[p, c, q] = Q[q, c*128+p]; dim1 = d-chunk for DoubleRow's
            # (p, t) contraction layout.
            qs = singles.tile([128, SHARD // 128, D], bf16, tag="qs", name="qs")
            q_re = q_ext.rearrange("(n p) d -> p n d", p=128)
            nc.gpsimd.dma_start(out=qs[:, 0:4, :], in_=q_re[:, 0:4, :])
            nc.gpsimd.dma_start(out=qs[:, 4:8, :], in_=q_re[:, 4:8, :])
            st_dt = bf16 if variant == "seqbf" else f8
            qt8 = singles.tile([128, 2, SHARD], st_dt, tag="qt8", name="qt8")
            for c in range(2):
                for g in range(SHARD // 512):
                    tr = (trp.tile([128, 512], bf16, tag="tr", name="tr")
                          if variant in ("seq", "seqp", "stag") else
                          stp.tile([128, 512], bf16, tag="st", name="tr"))
                    for j in range(4):
                        qi = g * 4 + j
                        nc.tensor.transpose(
                            tr[:, j * 128:(j + 1) * 128],
                            qs[:, qi, c * 128:(c + 1) * 128],
                            ident,
                        )
                    nc.vector.tensor_copy(
                        qt8[:, c, g * 512:(g + 1) * 512], tr[:]
                    )

            # ---- K: load (cast to fp8) + PE-transpose into KT [128,2,S] ----
            kt8 = singles.tile([128, 2, S], st_dt, tag="kt8", name="kt8")
            k_all = k_ext.rearrange("(b p) d -> p b d", p=128)
            groups = [(0, 4), (4, 4)] + [(8 * t, 8) for t in range(1, 8)]
            for b_start, nb in groups:
                ks = kstage.tile([128, 8, D], bf16, tag="ks", name="ks")
                nc.gpsimd.dma_start(
                    out=ks[:, 0:nb, :],
                    in_=k_all[:, b_start:b_start + nb, :],
                )
                for c in range(2):
                    for g in range(nb // 4):
                        tr = (trp.tile([128, 512], bf16, tag="tr", name="tr")
                          if variant in ("seq", "seqp", "stag") else
                          stp.tile([128, 512], bf16, tag="st", name="tr"))
                        for j in range(4):
                            n = g * 4 + j
                            nc.tensor.transpose(
                                tr[:, j * 128:(j + 1) * 128],
                                ks[:, n, c * 128:(c + 1) * 128],
                                ident,
                            )
                        b0 = b_start + g * 4
                        nc.vector.tensor_copy(
                            kt8[:, c, b0 * 128:(b0 + 4) * 128], tr[:]
                        )

            # ---- V: load (cast) with a ones-column appended ----
            vo = []
            v_re = v_ext.rearrange("(t n p) d -> t p n d", p=128, n=4)
            for t in range(S // 512):
                vt = singles.tile([128, 4, D + 1], bf16, tag=f"vo{t}", name=f"vo{t}")
                nc.vector.memset(vt[:, :, D:D + 1], 1.0)
                nc.gpsimd.dma_start(out=vt[:, :, 0:D], in_=v_re[t])
                vo.append(vt)

            # ---- main: ST stream interleaved with kb-major PV chains ----
            # PV for key block kb is emitted LAG key-block-pairs after its
            # exp, so the PE always has PV matmuls to run between ST
            # matmuls instead of stalling at the exp production rate.
            pts = {0: [], 1: []}
            chains = {}

            def st_step(qc, kbp):
                st = stp.tile([128, 1024], f32, tag="st", name="st")
                ptag = (f"pt{qc}_{kbp}" if variant in ("seqp", "stag")
                        else f"pt{kbp}")
                pt = ptp.tile([128, 1024], bf16, tag=ptag, name=ptag)
                for half in range(2):
                    kb = kbp * 2 + half
                    if variant == "seqbf":
                        for c in range(2):
                            nc.tensor.matmul(
                                st[:, half * 512:(half + 1) * 512],
                                kt8[:, c, kb * 128:(kb + 1) * 128],
                                qt8[:, c, qc * 512:(qc + 1) * 512],
                                start=(c == 0),
                                stop=(c == 1),
                            )
                    else:
                        nc.tensor.matmul(
                            st[:, half * 512:(half + 1) * 512],
                            kt8[:, :, kb * 128:(kb + 1) * 128],
                            qt8[:, :, qc * 512:(qc + 1) * 512],
                            start=True,
                            stop=True,
                            perf_mode=mybir.MatmulPerfMode.DoubleRow,
                        )
                nc.scalar.activation(pt[:], st[:], AF.Exp, scale=SCALE)
                pts[qc].append(pt)

            def pv_kb(qc, kb):
                for qt_i in range(NQT):
                    if kb == 0:
                        chains[(qc, qt_i)] = pvp.tile(
                            [128, D + 1], f32, tag="pv", name="pv"
                        )
                    pv = chains[(qc, qt_i)]
                    col0 = (kb % 2) * 512 + qt_i * 128
                    nc.tensor.matmul(
                        pv[:],
                        pts[qc][kb // 2][:, col0:col0 + 128],
                        vo[kb // 4][:, kb % 4, :],
                        start=(kb == 0),
                        stop=(kb == NKB - 1),
                    )
                if kb == NKB - 1:
                    for qt_i in range(NQT):
                        pv = chains[(qc, qt_i)]
                        rcp = outp.tile([128, 1], f32, tag="rcp", name="rcp")
                        nc.vector.reciprocal(rcp[:], pv[:, D:D + 1])
                        ot = outp.tile([128, D], f32, tag="ot", name="ot")
                        nc.vector.tensor_scalar_mul(ot[:], pv[:, 0:D], rcp[:])
                        row0 = qc * 512 + qt_i * 128
                        nc.sync.dma_start(
                            out=out_ext[row0:row0 + 128, :], in_=ot[:]
                        )

            if variant == "ilv":
                LAG = 2
                BURST = 8  # drain PV in bursts, limits LDW mode flips
                queue = []
                started = 0
                for qc in range(NQC):
                    for kbp in range(NKB // 2):
                        st_step(qc, kbp)
                        started += 1
                        queue.append((qc, 2 * kbp))
                        queue.append((qc, 2 * kbp + 1))
                        if started > LAG and len(queue) >= 2 * LAG + BURST:
                            for _ in range(BURST):
                                pv_kb(*queue.pop(0))
                while queue:
                    pv_kb(*queue.pop(0))
            elif variant == "stag":
                # qc0: plain ST phase. Then steps interleaving one ST(qc1)
                # pair with 8 contiguous PV(qc0) chain matmuls (chains stay
                # qt-major / single-bank). Finally PV(qc1) qt-major.
                def pv_chain_part(qc, qt_i, kb0, kb1):
                    if kb0 == 0:
                        chains[(qc, qt_i)] = pvp.tile(
                            [128, D + 1], f32, tag="pv", name="pv"
                        )
                    pv = chains[(qc, qt_i)]
                    for kb in range(kb0, kb1):
                        col0 = (kb % 2) * 512 + qt_i * 128
                        nc.tensor.matmul(
                            pv[:],
                            pts[qc][kb // 2][:, col0:col0 + 128],
                            vo[kb // 4][:, kb % 4, :],
                            start=(kb == 0),
                            stop=(kb == NKB - 1),
                        )
                    if kb1 == NKB:
                        rcp = outp.tile([128, 1], f32, tag="rcp", name="rcp")
                        nc.vector.reciprocal(rcp[:], pv[:, D:D + 1])
                        ot = outp.tile([128, D], f32, tag="ot", name="ot")
                        nc.vector.tensor_scalar_mul(ot[:], pv[:, 0:D], rcp[:])
                        row0 = qc * 512 + qt_i * 128
                        nc.sync.dma_start(
                            out=out_ext[row0:row0 + 128, :], in_=ot[:]
                        )

                for kbp in range(NKB // 2):
                    st_step(0, kbp)
                for step in range(32):
                    st_step(1, step)
                    qt_i, seg = divmod(step, 8)
                    pv_chain_part(0, qt_i, seg * 8, seg * 8 + 8)
                for qt_i in range(NQT):
                    pv_chain_part(1, qt_i, 0, NKB)
            elif variant == "seq4":
                # per-chunk: all ST, then kb-major PV (all 4 chains advance
                # together so pt slots release early for the next chunk)
                for qc in range(NQC):
                    for kbp in range(NKB // 2):
                        st_step(qc, kbp)
                    for kb in range(NKB):
                        pv_kb(qc, kb)
            else:
                # sequential: all ST of a chunk, then qt-major PV chains
                for qc in range(NQC):
                    for kbp in range(NKB // 2):
                        st_step(qc, kbp)
                    for qt_i in range(NQT):
                        pv = pvp.tile([128, D + 1], f32, tag="pv", name="pv")
                        for kb in range(NKB):
                            col0 = (kb % 2) * 512 + qt_i * 128
                            nc.tensor.matmul(
                                pv[:],
                                pts[qc][kb // 2][:, col0:col0 + 128],
                                vo[kb // 4][:, kb % 4, :],
                                start=(kb == 0),
                                stop=(kb == NKB - 1),
                            )
                        rcp = outp.tile([128, 1], f32, tag="rcp", name="rcp")
                        nc.vector.reciprocal(rcp[:], pv[:, D:D + 1])
                        ot = outp.tile([128, D], f32, tag="ot", name="ot")
                        nc.vector.tensor_scalar_mul(ot[:], pv[:, 0:D], rcp[:])
                        row0 = qc * 512 + qt_i * 128
                        nc.sync.dma_start(
                            out=out_ext[row0:row0 + 128, :], in_=ot[:]
                        )


def _get_nc(repeat=1, variant="stag"):
    key = f"nc{repeat}-{variant}"
    if key not in _CACHE:
        _CACHE[key] = _build(repeat, variant)
    return _CACHE[key]


def run(inputs, trace=False):
    """Run on 8 cores; returns (full_output, BassKernelResults)."""
    from concourse.bass_utils import run_bass_kernel_spmd

    Q = np.ascontiguousarray(np.asarray(inputs["Q"], dtype=np.float32))
    K = np.ascontiguousarray(np.asarray(inputs["K"], dtype=np.float32))
    V = np.ascontiguousarray(np.asarray(inputs["V"], dtype=np.float32))

    nc = _get_nc()
    in_maps = [
        {"Q": Q[i * SHARD:(i + 1) * SHARD], "K": K, "V": V}
        for i in range(N_CORES)
    ]
    res = run_bass_kernel_spmd(
        nc, in_maps, core_ids=list(range(N_CORES)), trace=trace
    )
    out = np.concatenate([res.results[i]["out"] for i in range(N_CORES)], axis=0)
    return out, res


def kernel(**inputs) -> np.ndarray:
    import time

    last_err = None
    for attempt in range(3):
        try:
            out, _ = run(inputs, trace=False)
            return out
        except Exception as e:  # transient axon/device wedge - retry
            last_err = e
            time.sleep(15 * (attempt + 1))
    raise last_err
